# revision 31
# baseline (speedup 1.0000x reference)
"""Trainium2 Bass kernel for nn_LocalMixtureNN (self-contained).

Strategy
--------
Pure data parallel over batch: 8 cores x 4 batches. Within a core the 128
(s, b) positions live on the 128 SBUF partitions (pos = s*BL + b, s-major so
n-gram window shifts are partition shifts by BL*k, realized as matmuls with
constant shift matrices).

v3 notes (92.5us, vs 161us v1 baseline):
- All matmuls are single-pass: bf16 where rounding is tolerable (LSTM
  gates/GX/final MLP/mixing; in-kernel casts), float32r elsewhere. fp32
  2-pass matmuls are never used.
- Both softmaxes use exp(x) = (1+tanh(x/2))/(1-tanh(x/2)) so the Exp table
  is never needed; all Sin ACTs are forced (fake deps) before the first
  Tanh so exactly two ACT table loads occur, both off the critical path.
- LSTM gate pre-activations live in one persistent PSUM bank, zero-filled
  by a single start=True matmul (start=True clears has_written bank-wide,
  so it must happen exactly once per bank); GX matmuls and the per-step
  recurrent-gate matmuls all accumulate with start=False. ACT reads the
  PSUM directly; steady-state step cadence ~1.66us.
- The measurement pipeline runs in two 64-position chunks: chunk 0
  (steps 0..15) is interleaved into LSTM steps 18..30, only chunk 1 forms
  the tail. Chunk c owns partition range [64c, 64c+64) of shared tiles
  (matmul PSUM outputs must start at partition 0; DVE ops may mix
  32-aligned partition bases between operands).
- The real/imag tensor-product combines ride PSUM accumulation of paired
  transposes; rsqrt is a bitcast magic seed + 2 Newton iterations; the
  1/kn^2 and 1/(n0*n1)^2 scales are plain DVE reciprocals applied late.
"""

import numpy as np

try:
    import concourse  # noqa: F401
except ImportError:  # pragma: no cover
    import sys

    sys.path.insert(0, "/opt/trn_rl_repo")

from contextlib import ExitStack

import concourse.bass as bass
import concourse.bacc as bacc
import concourse.tile as tile
import concourse.mybir as mybir

dt = mybir.dt
F32 = dt.float32
R32 = dt.float32r
BF16 = dt.bfloat16
I32 = dt.int32
AF = mybir.ActivationFunctionType
OP = mybir.AluOpType

NCORES = 8
B, S = 32, 32
BL = B // NCORES  # 4 batches per core
NPOS = S * BL  # 128 positions per core, pos = s*BL + b
VOCAB, EMBD, AUDIO, HID = 5000, 300, 74, 128
D1 = 16
D2 = 16
DIM = D1 * D2  # 256
UNITS, CELL = 128, 64
HALF_PI = float(np.pi / 2)
MAGIC = 0x5F3759DF

# pack0r: matmul constants (all float32r)
PACK0R = {}
_c = 0
for _name, _ncol in (
    ("ident", 128), ("s1t", 128), ("s2t", 128), ("fw1t", 64), ("fw2t", 64),
    ("wlin", 16), ("w1b", 16), ("w2b", 16), ("w3b", 16), ("fw3t", 1),
):
    PACK0R[_name] = (_c, _ncol)
    _c += _ncol
PACK0R_COLS = _c  # 577
# pack0f: DVE-side constants (float32)
PACK0F = {}
_c = 0
for _name, _ncol in (
    ("blin", 16), ("mwb", 2), ("fb1", 1), ("fb2", 1), ("fb3", 1),
):
    PACK0F[_name] = (_c, _ncol)
    _c += _ncol
PACK0F_COLS = _c  # 21
# pack2: recurrent weights + measurement kernel
PACK2 = {}
_c = 0
for _name, _ncol in (("whht", 512), ("mr", 256), ("mi", 256)):
    PACK2[_name] = (_c, _ncol)
    _c += _ncol
PACK2_COLS = _c  # 1024


def _outer_bcast(ap, n):
    """AP reading t[p, j] broadcast over a new leading free dim of size n."""
    return bass.AP(tensor=ap.tensor, offset=ap.offset, ap=[ap.ap[0], [0, n], ap.ap[1]])


def build_nc():
    nc = bacc.Bacc("TRN2", target_bir_lowering=False, debug=False)

    # ---------------- DRAM tensors (per-core inputs) ----------------
    wi_d = nc.dram_tensor("wi", [NPOS, 1], I32, kind="ExternalInput")
    au_d = nc.dram_tensor("au", [NPOS, AUDIO], R32, kind="ExternalInput")
    lute_d = nc.dram_tensor("lute", [VOCAB, EMBD], R32, kind="ExternalInput")
    lutp_d = nc.dram_tensor("lutp", [VOCAB, D1 + D2], F32, kind="ExternalInput")
    pack0r_d = nc.dram_tensor("pack0r", [128, PACK0R_COLS], R32, kind="ExternalInput")
    pack0f_d = nc.dram_tensor("pack0f", [128, PACK0F_COLS], F32, kind="ExternalInput")
    pack1_d = nc.dram_tensor("pack1", [128, 1536], R32, kind="ExternalInput")
    pack2_d = nc.dram_tensor("pack2", [128, PACK2_COLS], R32, kind="ExternalInput")
    y_d = nc.dram_tensor("y", [BL, 1], F32, kind="ExternalOutput")

    with tile.TileContext(nc) as tc, ExitStack() as ctx:
        cpool = ctx.enter_context(tc.tile_pool(name="const", bufs=1))
        wpool = ctx.enter_context(tc.tile_pool(name="work", bufs=2))
        npool = ctx.enter_context(tc.tile_pool(name="newton", bufs=2))
        lpool = ctx.enter_context(tc.tile_pool(name="lstm", bufs=3))
        gpall = ctx.enter_context(tc.tile_pool(name="gpall", bufs=1, space="PSUM"))
        ps_t = ctx.enter_context(tc.tile_pool(name="pst", bufs=3, space="PSUM"))
        ps_m = ctx.enter_context(tc.tile_pool(name="psm", bufs=2, space="PSUM"))
        ps_j = ctx.enter_context(tc.tile_pool(name="psj", bufs=1, space="PSUM"))

        # ---------------- bulk loads (criticality order) ----------------
        # wi rides alone at the head of the sync ring so the gathers can
        # start as early as possible; the big weight packs share the scalar
        # ring; audio trails the gathers on the gpsimd ring.
        wi = cpool.tile([NPOS, 1], I32, tag="wi")
        nc.sync.dma_start(wi[:], wi_d[:])
        pack1 = cpool.tile([128, 1536], R32, tag="pack1")
        nc.scalar.dma_start(pack1[:], pack1_d[:])
        gath_p = cpool.tile([NPOS, D1 + D2], F32, tag="gath_p")
        nc.gpsimd.indirect_dma_start(
            out=gath_p[:], out_offset=None, in_=lutp_d[:],
            in_offset=bass.IndirectOffsetOnAxis(ap=wi[:, 0:1], axis=0),
        )
        gath_e = cpool.tile([NPOS, EMBD], R32, tag="gath_e")
        nc.gpsimd.indirect_dma_start(
            out=gath_e[:], out_offset=None, in_=lute_d[:],
            in_offset=bass.IndirectOffsetOnAxis(ap=wi[:, 0:1], axis=0),
        )
        pack0r = cpool.tile([128, PACK0R_COLS], R32, tag="pack0r")
        nc.scalar.dma_start(pack0r[:], pack0r_d[:])
        pack2 = cpool.tile([128, PACK2_COLS], R32, tag="pack2")
        nc.scalar.dma_start(pack2[:], pack2_d[:])
        pack0f = cpool.tile([128, PACK0F_COLS], F32, tag="pack0f")
        nc.sync.dma_start(pack0f[:], pack0f_d[:])
        au = cpool.tile([NPOS, AUDIO], R32, tag="au")
        nc.gpsimd.dma_start(au[:], au_d[:])

        def p0r(name, nrow=128):
            c0, ncol = PACK0R[name]
            return pack0r[0:nrow, c0 : c0 + ncol]

        def p0f(name, nrow=128):
            c0, ncol = PACK0F[name]
            return pack0f[0:nrow, c0 : c0 + ncol]

        def p2(name, nrow=128):
            c0, ncol = PACK2[name]
            return pack2[0:nrow, c0 : c0 + ncol]

        wihta = pack1[:, 0:512]
        wihtb = pack1[:, 512:1024]
        wihtc = pack1[0:45, 1024:1536]
        whht = p2("whht")
        mr = p2("mr")
        mi = p2("mi")
        ident = p0r("ident")
        s1t = p0r("s1t")
        s2t = p0r("s2t")
        fw1t = p0r("fw1t")
        fw2t = p0r("fw2t", 64)
        wlin = p0r("wlin")
        w1b = p0r("w1b", 75)
        w2b = p0r("w2b", 17)
        w3b = p0r("w3b", 17)
        fw3t = p0r("fw3t", 64)
        blin_bc = p0f("blin")
        mwb = p0f("mwb")
        fb1 = p0f("fb1", 64)
        fb2 = p0f("fb2", 64)
        fb3 = p0f("fb3", 1)

        # zero hidden state for step 0 + audio layout tiles (ones rows)
        hz = cpool.tile([HID, BL], BF16, tag="hz")
        nc.gpsimd.memset(hz[:], 0.0)
        auT = cpool.tile([96, NPOS], R32, tag="auT")
        nc.gpsimd.memset(auT[:].bitcast(F32), 1.0)
        a1b = cpool.tile([32, NPOS], R32, tag="a1b")
        nc.gpsimd.memset(a1b[:].bitcast(F32), 1.0)
        a2b = cpool.tile([32, NPOS], R32, tag="a2b")
        nc.gpsimd.memset(a2b[:].bitcast(F32), 1.0)

        def rsqrt_magic(eng, x_ap, tag, iters=2):
            """r ~= rsqrt(x) via bitcast seed + Newton; x_ap is (128,1)."""
            vi = npool.tile([128, 1], I32, tag=f"{tag}_vi")
            eng.tensor_scalar(
                out=vi[:], in0=x_ap.bitcast(I32), scalar1=1, scalar2=None,
                op0=OP.logical_shift_right,
            )
            si = npool.tile([128, 1], I32, tag=f"{tag}_si")
            eng.tensor_scalar(
                out=si[:], in0=vi[:], scalar1=-1, scalar2=MAGIC,
                op0=OP.mult, op1=OP.add,
            )
            hx = npool.tile([128, 1], F32, tag=f"{tag}_hx")
            eng.tensor_scalar(
                out=hx[:], in0=x_ap, scalar1=-0.5, scalar2=None, op0=OP.mult
            )
            cur = si[:].bitcast(F32)
            for i in range(iters):
                z = npool.tile([128, 1], F32, tag=f"{tag}_z")
                eng.scalar_tensor_tensor(
                    out=z[:], in0=cur, scalar=hx[:, 0:1], in1=cur,
                    op0=OP.mult, op1=OP.mult,
                )
                nxt = npool.tile([128, 1], F32, tag=f"{tag}_r")
                eng.scalar_tensor_tensor(
                    out=nxt[:], in0=z[:], scalar=1.5, in1=cur,
                    op0=OP.add, op1=OP.mult,
                )
                cur = nxt[:]
            return cur

        # ---------------- trig ----------------
        PI = float(np.pi)
        TWO_PI = float(2 * np.pi)
        ph0g = gath_p[:, 0:D1]
        ph1g = gath_p[:, D1 : D1 + D2]

        def wrap3(src_ap, width, tag):
            cur = src_ap
            for p in range(3):
                t = wpool.tile([128, width], F32, tag=f"{tag}_w")
                nc.vector.add_range_wrap(
                    out=t[:], in_=cur, shift=0.0, bound=PI, period=TWO_PI
                )
                cur = t[:]
            return cur

        ph0w = wrap3(ph0g, D1, "wr0")
        ph1w = wrap3(ph1g, D2, "wr1")
        ph0c = wpool.tile([128, D1], F32, tag="ca0")
        nc.vector.add_range_wrap(
            out=ph0c[:], in_=ph0w, shift=HALF_PI, bound=PI, period=TWO_PI
        )
        ph1c = wpool.tile([128, D2], F32, tag="ca1")
        nc.vector.add_range_wrap(
            out=ph1c[:], in_=ph1w, shift=HALF_PI, bound=PI, period=TWO_PI
        )
        cos0 = cpool.tile([NPOS, D1], F32, tag="cos0")
        nc.scalar.activation(out=cos0[:], in_=ph0c[:], func=AF.Sin)
        sin0 = cpool.tile([NPOS, D1], F32, tag="sin0")
        nc.scalar.activation(out=sin0[:], in_=ph0w, func=AF.Sin)
        cos1 = cpool.tile([NPOS, D2], F32, tag="cos1")
        nc.scalar.activation(out=cos1[:], in_=ph1c[:], func=AF.Sin)
        sin1 = cpool.tile([NPOS, D2], F32, tag="sin1")
        nc.scalar.activation(out=sin1[:], in_=ph1w, func=AF.Sin)

        # modality softmax over 2: mw0 = sigmoid(a-b) via tanh.
        # dm is given artificial deps on all four Sin outputs so every Sin
        # is scheduled before the first Tanh: exactly two ACT table loads
        # (trig at entry, then the tanh set), both off the critical path.
        dm = wpool.tile([128, 1], F32, tag="dm")
        nc.vector.tensor_tensor(
            out=dm[:], in0=mwb[:, 0:1], in1=mwb[:, 1:2], op=OP.subtract
        )
        j1 = wpool.tile([128, 1], F32, tag="j1")
        nc.vector.scalar_tensor_tensor(
            out=j1[:], in0=cos0[:, 0:1], scalar=0.0, in1=sin0[:, 0:1],
            op0=OP.mult, op1=OP.mult,
        )
        j2 = wpool.tile([128, 1], F32, tag="j2")
        nc.vector.scalar_tensor_tensor(
            out=j2[:], in0=cos1[:, 0:1], scalar=0.0, in1=sin1[:, 0:1],
            op0=OP.mult, op1=OP.mult,
        )
        jj = wpool.tile([128, 1], F32, tag="jj")
        nc.vector.tensor_tensor(out=jj[:], in0=j1[:], in1=j2[:], op=OP.add)
        dm2 = wpool.tile([128, 1], F32, tag="dm2")
        nc.vector.scalar_tensor_tensor(
            out=dm2[:], in0=jj[:], scalar=1.0, in1=dm[:], op0=OP.mult, op1=OP.add
        )
        tmw = wpool.tile([128, 1], F32, tag="tmw")
        nc.scalar.activation(out=tmw[:], in_=dm2[:], func=AF.Tanh, scale=0.5)
        mw0 = cpool.tile([128, 1], F32, tag="mw0")
        nc.vector.tensor_scalar(
            out=mw0[:], in0=tmw[:], scalar1=0.5, scalar2=0.5, op0=OP.mult, op1=OP.add
        )
        mw1 = cpool.tile([128, 1], F32, tag="mw1")
        nc.vector.tensor_scalar(
            out=mw1[:], in0=tmw[:], scalar1=-0.5, scalar2=0.5, op0=OP.mult, op1=OP.add
        )

        # ---------------- embedding transpose ----------------
        embT0 = cpool.tile([128, NPOS], BF16, tag="embT0")
        pst = ps_t.tile([128, 128], F32, tag="pst")
        nc.tensor.transpose(pst[:].bitcast(R32), gath_e[:, 0:128], ident)
        nc.vector.tensor_copy(out=embT0[:], in_=pst[:])
        embT1 = cpool.tile([128, NPOS], BF16, tag="embT1")
        pst = ps_t.tile([128, 128], F32, tag="pst")
        nc.tensor.transpose(pst[:].bitcast(R32), gath_e[:, 128:256], ident)
        nc.vector.tensor_copy(out=embT1[:], in_=pst[:])
        embT2 = cpool.tile([64, NPOS], BF16, tag="embT2")
        nc.gpsimd.memset(embT2[:], 1.0)
        pst44 = ps_t.tile([44, 128], F32, tag="pst")
        nc.tensor.transpose(pst44[:].bitcast(R32), gath_e[:, 256:300], ident)
        nc.vector.tensor_copy(out=embT2[0:44, :], in_=pst44[:])

        ident_b = cpool.tile([128, 128], BF16, tag="ident_b")
        nc.vector.tensor_copy(out=ident_b[:], in_=ident)
        identn_b = cpool.tile([128, 128], BF16, tag="identn_b")
        nc.vector.tensor_scalar(
            out=identn_b[:], in0=ident, scalar1=-1.0, scalar2=None, op0=OP.mult
        )
        s1t_b = cpool.tile([128, 128], BF16, tag="s1t_b")
        nc.vector.tensor_copy(out=s1t_b[:], in_=s1t)
        s2t_b = cpool.tile([128, 128], BF16, tag="s2t_b")
        nc.vector.tensor_copy(out=s2t_b[:], in_=s2t)
        fw1t_b = cpool.tile([128, 64], BF16, tag="fw1t_b")
        nc.vector.tensor_copy(out=fw1t_b[:], in_=fw1t)
        fw2t_b = cpool.tile([64, 64], BF16, tag="fw2t_b")
        nc.vector.tensor_copy(out=fw2t_b[:], in_=fw2t)
        fw3t_b = cpool.tile([64, 1], BF16, tag="fw3t_b")
        nc.vector.tensor_copy(out=fw3t_b[:], in_=fw3t)
        wihta_b = cpool.tile([128, 512], BF16, tag="wihta_b")
        nc.vector.tensor_copy(out=wihta_b[:], in_=wihta)
        wihtb_b = cpool.tile([128, 512], BF16, tag="wihtb_b")
        nc.vector.tensor_copy(out=wihtb_b[:], in_=wihtb)
        wihtc_b = cpool.tile([45, 512], BF16, tag="wihtc_b")
        nc.vector.tensor_copy(out=wihtc_b[:], in_=wihtc)
        # ---------------- GX accumulation into persistent PSUM ----------------
        # gp_all[hid, s, gate, b] holds the gate pre-activations; GX matmuls
        # accumulate the input part, each LSTM step adds the recurrent part.
        gp_all = gpall.tile([HID, 4, S, BL], F32, tag="gp_all")
        GX_BLOCKS = ((0, 8), (8, 32))
        # one start=True matmul zero-fills the whole bank: a start=True
        # clears has_written bank-wide, so it must happen exactly once here
        # and every later write into this bank accumulates (start=False).
        zmov = cpool.tile([128, 4 * S * BL], R32, tag="zmov")
        nc.gpsimd.memset(zmov[:].bitcast(F32), 0.0)
        nc.tensor.matmul(
            gp_all[:, :, :, :], ident, zmov[:],
            start=True, stop=False, skip_group_check=True,
        )

        def emit_gx(blk, g):
            s0, s1 = GX_BLOCKS[blk]
            p0, p1 = s0 * BL, s1 * BL
            gsl = slice(HID * g, HID * (g + 1))
            out_ap = gp_all[:, g, s0:s1, :]
            nc.tensor.matmul(
                out_ap, wihta_b[:, gsl], embT0[:, p0:p1],
                start=False, stop=False, skip_group_check=True,
            )
            nc.tensor.matmul(
                out_ap, wihtb_b[:, gsl], embT1[:, p0:p1],
                start=False, stop=False, skip_group_check=True,
            )
            nc.tensor.matmul(
                out_ap, wihtc_b[:, gsl], embT2[0:45, p0:p1],
                start=False, stop=False, skip_group_check=True,
            )

        for g in range(4):
            emit_gx(0, g)

        # ---------------- LSTM step emission ----------------
        H2 = cpool.tile([HID, NPOS], BF16, tag="H2")
        whht_b = cpool.tile([HID, 512], BF16, tag="whht_b")
        nc.vector.tensor_copy(out=whht_b[:], in_=whht)
        wlin_b = cpool.tile([128, D1], BF16, tag="wlin_b")
        nc.vector.tensor_copy(out=wlin_b[:], in_=wlin)
        state = {"c2": None}
        # scratch bank for PE-warming matmuls (keeps the HAM clock gate at
        # 2.4GHz during the LSTM; results are never read)
        junk_ps = ps_j.tile([128, 448], F32, tag="junk_ps")

        def pe_warm(n=3):
            for _ in range(n):
                nc.tensor.matmul(
                    junk_ps[:, 0:448], ident, zmov[:, 0:448],
                    start=True, stop=True, skip_group_check=True,
                )

        def emit_step(s):
            hprev = hz[:] if s == 0 else H2[:, (s - 1) * BL : s * BL]
            for g in range(3):
                nc.tensor.matmul(
                    gp_all[:, g, s, :], whht_b[:, HID * g : HID * (g + 1)], hprev,
                    start=False, stop=True, skip_group_check=True,
                )
            g3 = lpool.tile([128, 3, BL], F32, tag="g3")
            nc.scalar.activation(out=g3[:], in_=gp_all[:, 0:3, s, :], func=AF.Tanh)
            nc.tensor.matmul(
                gp_all[:, 3, s, :], whht_b[:, HID * 3 : HID * 4], hprev,
                start=False, stop=True, skip_group_check=True,
            )
            tot = lpool.tile([128, BL], F32, tag="tot")
            nc.scalar.activation(out=tot[:], in_=gp_all[:, 3, s, :], func=AF.Tanh)
            ti, tf, tg = g3[:, 0, :], g3[:, 1, :], g3[:, 2, :]
            q2 = lpool.tile([128, BL], F32, tag="q2")
            nc.vector.scalar_tensor_tensor(
                out=q2[:], in0=ti, scalar=1.0, in1=tg, op0=OP.add, op1=OP.mult
            )
            if s == 0:
                c2 = q2
            else:
                q1 = lpool.tile([128, BL], F32, tag="q1")
                nc.vector.scalar_tensor_tensor(
                    out=q1[:], in0=tf, scalar=1.0, in1=state["c2"][:],
                    op0=OP.add, op1=OP.mult,
                )
                c2 = lpool.tile([128, BL], F32, tag="c2")
                nc.vector.scalar_tensor_tensor(
                    out=c2[:], in0=q1[:], scalar=0.5, in1=q2[:],
                    op0=OP.mult, op1=OP.add,
                )
            th = lpool.tile([128, BL], F32, tag="th")
            nc.scalar.activation(out=th[:], in_=c2[:], func=AF.Tanh, scale=0.5)
            nc.vector.scalar_tensor_tensor(
                out=H2[:, s * BL : (s + 1) * BL], in0=tot[:], scalar=1.0, in1=th[:],
                op0=OP.add, op1=OP.mult,
            )
            state["c2"] = c2

        # ---------------- fillers (emitted between LSTM steps) ----------------
        post = {}

        def audio_mm1():
            psau = ps_t.tile([AUDIO, NPOS], F32, tag="pst")
            nc.tensor.transpose(psau[:].bitcast(R32), au[:], ident)
            nc.vector.tensor_copy(out=auT[0:AUDIO, :], in_=psau[:])

        def audio_mm2():
            psm = ps_t.tile([D2, NPOS], F32, tag="pst")
            nc.tensor.matmul(psm[:], w1b, auT[0 : AUDIO + 1, :], start=True, stop=True)
            nc.vector.tensor_scalar(
                out=a1b[0:D2, :], in0=psm[:], scalar1=0.0, scalar2=None, op0=OP.max
            )

        def audio_mm3():
            psm = ps_t.tile([D2, NPOS], F32, tag="pst")
            nc.tensor.matmul(psm[:], w2b, a1b[0 : D2 + 1, :], start=True, stop=True)
            nc.vector.tensor_scalar(
                out=a2b[0:D2, :], in0=psm[:], scalar1=0.0, scalar2=None, op0=OP.max
            )

        amp1T = cpool.tile([D2, NPOS], R32, tag="amp1T")
        amp1 = cpool.tile([NPOS, D2], F32, tag="amp1")

        def audio_mm4():
            psm = ps_t.tile([D2, NPOS], F32, tag="pst")
            nc.tensor.matmul(psm[:], w3b, a2b[0 : D2 + 1, :], start=True, stop=True)
            nc.vector.tensor_scalar(
                out=amp1T[:], in0=psm[:], scalar1=0.0, scalar2=None, op0=OP.max
            )

        def audio_tr():
            ps16 = ps_t.tile([128, D2], F32, tag="pst")
            nc.tensor.transpose(ps16[:].bitcast(R32), amp1T[:], ident[0:D2, 0:D2])
            nc.vector.tensor_copy(out=amp1[:], in_=ps16[:])

        s1x = cpool.tile([128, 1], F32, tag="s1x")
        wpart = cpool.tile([128, 1], F32, tag="wpart")
        n1chain = {}

        def n1_a():
            junk16 = wpool.tile([128, D2], F32, tag="junk16")
            nc.vector.scalar_tensor_tensor(
                out=junk16[:], in0=amp1[:], scalar=1.0, in1=amp1[:],
                op0=OP.mult, op1=OP.mult, accum_out=s1x[:, 0:1],
            )

        def n1_b():
            n1chain["r"] = rsqrt_magic(nc.vector, s1x[:], "n1")

        def n1_c():
            n1 = wpool.tile([128, 1], F32, tag="n1v")
            nc.vector.tensor_scalar(
                out=n1[:], in0=s1x[:], scalar1=n1chain["r"][:, 0:1], scalar2=None,
                op0=OP.mult,
            )
            nc.vector.tensor_scalar(
                out=wpart[:], in0=n1[:], scalar1=mw1[:, 0:1], scalar2=None, op0=OP.mult
            )

        rkn2 = cpool.tile([128, 1], F32, tag="rkn2")

        def kn_chain():
            junk256 = wpool.tile([128, DIM], F32, tag="junk256")
            skr = wpool.tile([128, 1], F32, tag="skr")
            nc.vector.scalar_tensor_tensor(
                out=junk256[:], in0=mr, scalar=1.0, in1=mr,
                op0=OP.mult, op1=OP.mult, accum_out=skr[:, 0:1],
            )
            ski = wpool.tile([128, 1], F32, tag="ski")
            nc.vector.scalar_tensor_tensor(
                out=junk256[:], in0=mi, scalar=1.0, in1=mi,
                op0=OP.mult, op1=OP.mult, accum_out=ski[:, 0:1],
            )
            kx = wpool.tile([128, 1], F32, tag="kx")
            nc.gpsimd.tensor_tensor(out=kx[:], in0=skr[:], in1=ski[:], op=OP.add)
            nc.vector.reciprocal(out=rkn2[:], in_=kx[:])

        vrviT = []

        def vrvi(k):
            t = cpool.tile([128, 2 * UNITS], BF16, tag=f"vrviT{k}")
            for half, srct in ((0, mr), (1, mi)):
                pstv = ps_t.tile([128, 128], F32, tag="pst")
                nc.tensor.transpose(
                    pstv[:].bitcast(R32), srct[:, 128 * k : 128 * (k + 1)], ident
                )
                nc.scalar.copy(out=t[:, 128 * half : 128 * (half + 1)], in_=pstv[:])
            vrviT.append(t)

        r1v = cpool.tile([NPOS, D2], F32, tag="r1v")
        i1v = cpool.tile([NPOS, D2], F32, tag="i1v")
        uv = cpool.tile([NPOS, D2], F32, tag="uv")
        vv = cpool.tile([NPOS, D2], F32, tag="vv")

        def branch1():
            nc.gpsimd.tensor_tensor(out=r1v[:], in0=amp1[:], in1=cos1[:], op=OP.mult)
            nc.gpsimd.tensor_tensor(out=i1v[:], in0=amp1[:], in1=sin1[:], op=OP.mult)
            nc.gpsimd.tensor_tensor(out=uv[:], in0=r1v[:], in1=i1v[:], op=OP.subtract)
            nc.gpsimd.tensor_tensor(out=vv[:], in0=r1v[:], in1=i1v[:], op=OP.add)

        # ---------------- post-phase chunks (64 positions each) ----------
        # Chunk 0 (positions 0..63, steps 0..15) runs as fillers inside the
        # LSTM; chunk 1 forms the tail. Shared tiles, chunk c owns the
        # 32-aligned partition range [64c, 64c+64).
        amp0 = cpool.tile([NPOS, D1], F32, tag="amp0")
        s0x = cpool.tile([128, 1], F32, tag="s0x")
        csqt = cpool.tile([128, 1], F32, tag="csqt")
        r0v = cpool.tile([NPOS, D1], F32, tag="r0v")
        i0v = cpool.tile([NPOS, D1], F32, tag="i0v")
        tmpA = cpool.tile([NPOS, DIM], BF16, tag="tmpA")
        tmpB = cpool.tile([NPOS, DIM], BF16, tag="tmpB")
        tmpC = cpool.tile([NPOS, DIM], BF16, tag="tmpC")
        tmpD = cpool.tile([NPOS, DIM], BF16, tag="tmpD")
        rimT = cpool.tile([128, 4, NPOS], BF16, tag="rimT")
        rt_sb = cpool.tile([NPOS, 2 * UNITS], F32, tag="rt_sb")
        u1 = cpool.tile([NPOS, UNITS], F32, tag="u1")
        u2 = cpool.tile([NPOS, UNITS], F32, tag="u2")
        sq1 = cpool.tile([NPOS, UNITS], F32, tag="sq1")
        sq2 = cpool.tile([NPOS, UNITS], F32, tag="sq2")
        msu = cpool.tile([NPOS, UNITS], F32, tag="msu")
        msr = cpool.tile([NPOS, UNITS], BF16, tag="msr")
        pq_ps = {}
        rt_ps = {}

        def PSL(c):
            return slice(64 * c, 64 * (c + 1))

        wlin_b2 = cpool.tile([128, D1], BF16, tag="wlin_b2")

        def wlin_gate():
            # fake dep on step 19's H2 so the scheduler cannot hoist chunk-0
            # work into the congested steps 17..19
            jz = wpool.tile([128, 1], F32, tag="jz")
            nc.vector.tensor_scalar(
                out=jz[:], in0=H2[:, 76:77], scalar1=0.0, scalar2=None, op0=OP.mult
            )
            nc.vector.tensor_scalar(
                out=wlin_b2[:], in0=wlin_b[:], scalar1=jz[:, 0:1], scalar2=None,
                op0=OP.add,
            )

        def pc_amp0(c):
            P = PSL(c)
            pa = ps_t.tile([64, D1], F32, tag="pst")
            nc.tensor.matmul(
                pa[:, :], H2[:, P], wlin_b2[:] if c == 0 else wlin_b[:],
                start=True, stop=True, skip_group_check=True,
            )
            nc.vector.tensor_tensor(
                out=amp0[P, :], in0=pa[:, :], in1=blin_bc[P, :], op=OP.add
            )

        def pc_norm(c):
            P = PSL(c)
            junk16b = wpool.tile([128, D1], F32, tag="junk16b")
            nc.vector.scalar_tensor_tensor(
                out=junk16b[P, :], in0=amp0[P, :], scalar=1.0, in1=amp0[P, :],
                op0=OP.mult, op1=OP.mult, accum_out=s0x[P, 0:1],
            )
            prod = wpool.tile([128, 1], F32, tag="prod")
            nc.vector.tensor_scalar(
                out=prod[P, :], in0=s0x[P, :], scalar1=s1x[P, 0:1], scalar2=1e-30,
                op0=OP.mult, op1=OP.add,
            )
            nc.vector.reciprocal(out=csqt[P, :], in_=prod[P, :])
            nc.vector.tensor_tensor(
                out=r0v[P, :], in0=amp0[P, :], in1=cos0[P, :], op=OP.mult
            )
            nc.vector.tensor_tensor(
                out=i0v[P, :], in0=amp0[P, :], in1=sin0[P, :], op=OP.mult
            )

        def pc_outer_re(c):
            P = PSL(c)
            eng = nc.gpsimd if c == 0 else nc.vector
            eng.tensor_tensor(
                out=tmpA[P, :].rearrange("p (i j) -> p i j", j=D2),
                in0=r0v[P, :].to_broadcast([64, D1, D2]),
                in1=_outer_bcast(uv[P, :], D1), op=OP.mult,
            )
            eng.tensor_tensor(
                out=tmpB[P, :].rearrange("p (i j) -> p i j", j=D2),
                in0=i0v[P, :].to_broadcast([64, D1, D2]),
                in1=_outer_bcast(vv[P, :], D1), op=OP.mult,
            )

        def pc_outer_im(c):
            P = PSL(c)
            nc.gpsimd.tensor_tensor(
                out=tmpC[P, :].rearrange("p (i j) -> p i j", j=D2),
                in0=r0v[P, :].to_broadcast([64, D1, D2]),
                in1=_outer_bcast(vv[P, :], D1), op=OP.mult,
            )
            nc.gpsimd.tensor_tensor(
                out=tmpD[P, :].rearrange("p (i j) -> p i j", j=D2),
                in0=i0v[P, :].to_broadcast([64, D1, D2]),
                in1=_outer_bcast(uv[P, :], D1), op=OP.mult,
            )

        def pc_trans(c, qs):
            # rimT[:, q, P] = transpose(tmpA_q) + transpose(tmpB_q): the real/
            # imag combines ride the PSUM accumulation of paired transposes.
            P = PSL(c)
            for q in qs:
                t1, t2 = (tmpA, tmpB) if q < 2 else (tmpC, tmpD)
                id2 = identn_b if q < 2 else ident_b
                qq = 128 * (q % 2)
                pstq = ps_t.tile([128, 64], F32, tag="pst")
                nc.tensor.matmul(
                    pstq[:], t1[P, qq : qq + 128], ident_b[P, P],
                    start=True, stop=False, skip_group_check=True,
                )
                nc.tensor.matmul(
                    pstq[:], t2[P, qq : qq + 128], id2[P, P],
                    start=False, stop=True, skip_group_check=True,
                )
                if c == 0:
                    nc.vector.tensor_copy(out=rimT[:, q, P], in_=pstq[:])
                else:
                    nc.scalar.copy(out=rimT[:, q, P], in_=pstq[:])

        def pc_pqrt(c, which):
            P = PSL(c)
            t = ps_m.tile([64, 2 * UNITS], F32, tag="psm")
            q0 = 0 if which == 0 else 2
            nc.tensor.matmul(
                t[:, :], rimT[:, q0, P], vrviT[0][:],
                start=True, stop=False, skip_group_check=True,
            )
            nc.tensor.matmul(
                t[:, :], rimT[:, q0 + 1, P], vrviT[1][:],
                start=False, stop=True, skip_group_check=True,
            )
            (pq_ps if which == 0 else rt_ps)[c] = t

        def pc_u(c):
            P = PSL(c)
            nc.vector.tensor_copy(out=rt_sb[P, :], in_=rt_ps[c][:, :])
            nc.vector.tensor_tensor(
                out=u1[P, :], in0=pq_ps[c][:, 0:UNITS],
                in1=rt_sb[P, UNITS : 2 * UNITS], op=OP.add,
            )
            nc.vector.tensor_tensor(
                out=u2[P, :], in0=rt_sb[P, 0:UNITS],
                in1=pq_ps[c][:, UNITS : 2 * UNITS], op=OP.subtract,
            )

        def pc_msr(c):
            P = PSL(c)
            nc.scalar.activation(out=sq1[P, :], in_=u1[P, :], func=AF.Square)
            nc.scalar.activation(out=sq2[P, :], in_=u2[P, :], func=AF.Square)
            nc.vector.tensor_tensor(
                out=msu[P, :], in0=sq1[P, :], in1=sq2[P, :], op=OP.add
            )
            nc.vector.tensor_scalar(
                out=msr[P, :], in0=msu[P, :], scalar1=csqt[P, 0:1], scalar2=None,
                op0=OP.mult,
            )

        # GX block 1 split into per-gate chunks to interleave with early steps
        post[1] = [lambda: emit_gx(1, 0), audio_mm1]
        post[2] = [lambda: emit_gx(1, 1), audio_mm2]
        post[3] = [lambda: emit_gx(1, 2), audio_mm3]
        post[4] = [lambda: emit_gx(1, 3), audio_mm4]
        post[5] = [audio_tr, lambda: vrvi(0)]
        post[6] = [lambda: vrvi(1), n1_a]
        post[7] = [n1_b]
        post[8] = [n1_c, kn_chain]
        post[9] = [branch1]
        post[19] = [wlin_gate]
        post[20] = [lambda: pc_amp0(0)]
        post[21] = [lambda: pc_norm(0)]
        post[22] = [lambda: pc_outer_re(0)]
        post[23] = [lambda: pc_outer_im(0)]
        post[24] = [lambda: pc_trans(0, (0,))]
        post[25] = [lambda: pc_trans(0, (1,))]
        post[26] = [lambda: pc_trans(0, (2,))]
        post[27] = [lambda: pc_trans(0, (3,))]
        post[28] = [lambda: pc_pqrt(0, 0)]
        post[29] = [lambda: pc_pqrt(0, 1)]
        post[30] = [lambda: pc_u(0)]
        post[31] = [lambda: pc_msr(0)]

        for s in range(S):
            emit_step(s)
            for fn in post.get(s, ()):
                fn()

        # ---------------- tail: chunk 1 + weight path ----------------
        pc_amp0(1)
        pc_norm(1)
        pc_outer_re(1)
        pc_outer_im(1)
        pc_trans(1, (0, 1, 2, 3))
        pc_pqrt(1, 0)
        pc_pqrt(1, 1)
        # weight / windowed-softmax path (needs s0x of both chunks); its DVE
        # work fills gaps while the PE does chunk-1 transposes + PQRT.
        r0n = rsqrt_magic(nc.vector, s0x[:], "n0")
        n0 = wpool.tile([128, 1], F32, tag="n0v")
        nc.vector.tensor_scalar(
            out=n0[:], in0=s0x[:], scalar1=r0n[:, 0:1], scalar2=None, op0=OP.mult
        )
        weight = wpool.tile([128, 2], BF16, tag="weight")
        nc.vector.scalar_tensor_tensor(
            out=weight[:], in0=n0[:].to_broadcast([128, 2]),
            scalar=mw0[:, 0:1], in1=wpart[:].to_broadcast([128, 2]),
            op0=OP.mult, op1=OP.add,
        )
        # windowed weights V = [w, shift4(w), shift8(w)] then softmax (tanh-exp)
        vp = ps_t.tile([128, 4], F32, tag="pst")
        nc.tensor.matmul(vp[:, 0:2], s1t_b[:], weight[:], start=True, stop=True)
        nc.tensor.matmul(vp[:, 2:4], s2t_b[:], weight[:], start=True, stop=True)
        v3 = wpool.tile([128, 3], F32, tag="v3")
        nc.gpsimd.tensor_copy(out=v3[:, 0:1], in_=weight[:, 0:1])
        nc.vector.tensor_copy(out=v3[:, 1:2], in_=vp[:, 0:1])
        nc.vector.tensor_copy(out=v3[:, 2:3], in_=vp[:, 2:3])
        vmax = wpool.tile([128, 1], F32, tag="vmax")
        nc.vector.tensor_reduce(
            out=vmax[:], in_=v3[:], axis=mybir.AxisListType.X, op=OP.max
        )
        v3s = wpool.tile([128, 3], F32, tag="v3s")
        nc.vector.tensor_scalar(
            out=v3s[:], in0=v3[:], scalar1=vmax[:, 0:1], scalar2=None, op0=OP.subtract
        )
        t3 = wpool.tile([128, 3], F32, tag="t3")
        nc.scalar.activation(out=t3[:], in_=v3s[:], func=AF.Tanh, scale=0.5)
        den3 = wpool.tile([128, 3], F32, tag="den3")
        nc.vector.tensor_scalar(
            out=den3[:], in0=t3[:], scalar1=-1.0, scalar2=1.0, op0=OP.mult, op1=OP.add
        )
        rec3 = wpool.tile([128, 3], F32, tag="rec3")
        nc.vector.reciprocal(out=rec3[:], in_=den3[:])
        e3 = wpool.tile([128, 3], F32, tag="e3")
        nc.vector.scalar_tensor_tensor(
            out=e3[:], in0=t3[:], scalar=1.0, in1=rec3[:], op0=OP.add, op1=OP.mult
        )
        esum = wpool.tile([128, 1], F32, tag="esum")
        nc.vector.tensor_reduce(
            out=esum[:], in_=e3[:], axis=mybir.AxisListType.X, op=OP.add
        )
        res = wpool.tile([128, 1], F32, tag="res")
        nc.vector.reciprocal(out=res[:], in_=esum[:])
        ww = wpool.tile([128, 3], F32, tag="ww")
        nc.vector.tensor_scalar(
            out=ww[:], in0=e3[:], scalar1=res[:, 0:1], scalar2=None, op0=OP.mult
        )
        pc_u(1)
        pc_msr(1)

        # n-gram mixing: m3 = ww0*M + ww1*shift4(M) + ww2*shift8(M)
        ps_ms = ps_m.tile([NPOS, 2 * UNITS], F32, tag="psm")
        nc.tensor.matmul(ps_ms[:, 0:UNITS], s1t_b[:], msr[:], start=True, stop=True)
        nc.tensor.matmul(ps_ms[:, UNITS : 2 * UNITS], s2t_b[:], msr[:], start=True, stop=True)
        t1m = wpool.tile([NPOS, UNITS], F32, tag="t1m")
        nc.vector.tensor_scalar(
            out=t1m[:], in0=ps_ms[:, 0:UNITS], scalar1=ww[:, 1:2], scalar2=None,
            op0=OP.mult,
        )
        m3a = wpool.tile([NPOS, UNITS], F32, tag="m3a")
        nc.vector.scalar_tensor_tensor(
            out=m3a[:], in0=ps_ms[:, UNITS : 2 * UNITS], scalar=ww[:, 2:3], in1=t1m[:],
            op0=OP.mult, op1=OP.add,
        )
        m3 = wpool.tile([NPOS, UNITS], F32, tag="m3")
        nc.vector.scalar_tensor_tensor(
            out=m3[:], in0=msr[:], scalar=ww[:, 0:1], in1=m3a[:],
            op0=OP.mult, op1=OP.add,
        )
        mmx = wpool.tile([NPOS, UNITS], R32, tag="mmx")
        nc.vector.tensor_tensor(out=mmx[:], in0=msr[:], in1=m3[:], op=OP.max)

        # max over positions per batch: transpose then reduce over s
        ps_mt = ps_t.tile([UNITS, NPOS], F32, tag="pst")
        nc.tensor.transpose(ps_mt[:].bitcast(R32), mmx[:], ident)
        featU = wpool.tile([UNITS, BL], F32, tag="featU")
        nc.vector.tensor_reduce(
            out=featU[:], in_=ps_mt[:].rearrange("p (s b) -> p b s", b=BL),
            axis=mybir.AxisListType.X, op=OP.max,
        )
        featT = wpool.tile([UNITS, BL], BF16, tag="featT")
        nc.vector.tensor_scalar(
            out=featT[:], in0=featU[:], scalar1=rkn2[:, 0:1], scalar2=None,
            op0=OP.mult,
        )

        # ---------------- final MLP ----------------
        y1p = ps_t.tile([CELL, BL], F32, tag="pst")
        nc.tensor.matmul(y1p[:], fw1t_b[:], featT[:], start=True, stop=True)
        y1 = wpool.tile([CELL, BL], BF16, tag="y1")
        nc.vector.tensor_scalar(
            out=y1[:], in0=y1p[:], scalar1=fb1[:, 0:1], scalar2=0.0,
            op0=OP.add, op1=OP.max,
        )
        y2p = ps_t.tile([CELL, BL], F32, tag="pst")
        nc.tensor.matmul(y2p[:], fw2t_b[:], y1[:], start=True, stop=True)
        y2 = wpool.tile([CELL, BL], BF16, tag="y2")
        nc.vector.tensor_scalar(
            out=y2[:], in0=y2p[:], scalar1=fb2[:, 0:1], scalar2=0.0,
            op0=OP.add, op1=OP.max,
        )
        y3p = ps_t.tile([1, BL], F32, tag="pst")
        nc.tensor.matmul(y3p[:], fw3t_b[:], y2[:], start=True, stop=True)
        ysb = wpool.tile([1, BL], F32, tag="ysb")
        nc.vector.tensor_scalar(
            out=ysb[:], in0=y3p[:], scalar1=fb3[0:1, 0:1], scalar2=None, op0=OP.add
        )
        nc.sync.dma_start(y_d[:], ysb[:])

    nc.compile()
    return nc


_NC = None


def _get_nc():
    global _NC
    if _NC is None:
        _NC = build_nc()
    return _NC


def _padrows(a, rows):
    out = np.zeros((rows, a.shape[1]), np.float32)
    out[: a.shape[0]] = a
    return out


def make_in_maps(inputs):
    """Host-side layout prep (value-preserving transforms only)."""
    f32 = np.float32
    wi_full = np.asarray(inputs["word_indexes"]).astype(np.int32)  # (B, S)
    au_full = np.asarray(inputs["audio"], dtype=f32)  # (B, S, A)
    lute = np.ascontiguousarray(np.asarray(inputs["lookup_table"], dtype=f32))
    lutp = np.ascontiguousarray(
        np.concatenate(
            [
                np.asarray(inputs["phase_tab0"], dtype=f32),
                np.asarray(inputs["phase_tab1"], dtype=f32),
            ],
            axis=1,
        )
    )
    w_ih = np.asarray(inputs["w_ih"], dtype=f32)
    w_hh = np.asarray(inputs["w_hh"], dtype=f32)
    b_ih = np.asarray(inputs["b_ih"], dtype=f32)
    b_hh = np.asarray(inputs["b_hh"], dtype=f32)
    w_lin = np.asarray(inputs["w_lin"], dtype=f32)
    b_lin = np.asarray(inputs["b_lin"], dtype=f32)

    # gate order [i, f, g, o]; sigma->tanh fold (x0.5 on i,f,o rows);
    # extra x0.5 on all w_hh entries for the doubled hidden state H2=2h.
    gsc = np.concatenate(
        [np.full(256, 0.5, f32), np.ones(128, f32), np.full(128, 0.5, f32)]
    )
    w_ih_p = w_ih * gsc[:, None]
    b_p = (b_ih + b_hh) * gsc
    wihT = np.concatenate([w_ih_p.T, b_p[None, :]], axis=0).astype(f32)  # (301,512)
    whht = (w_hh * gsc[:, None] * 0.5).T.astype(f32)  # (128, 512)

    pack1 = np.concatenate(
        [wihT[0:128], wihT[128:256], _padrows(wihT[256:301], 128)], axis=1
    )
    pack1 = np.ascontiguousarray(pack1, dtype=f32)

    w1 = np.asarray(inputs["w1"], dtype=f32)
    w2 = np.asarray(inputs["w2"], dtype=f32)
    w3 = np.asarray(inputs["w3"], dtype=f32)
    b1 = np.asarray(inputs["b1"], dtype=f32)
    b2 = np.asarray(inputs["b2"], dtype=f32)
    b3 = np.asarray(inputs["b3"], dtype=f32)
    parts0r = {
        "ident": np.eye(128, dtype=f32),
        "s1t": np.eye(128, k=4, dtype=f32).T,
        "s2t": np.eye(128, k=8, dtype=f32).T,
        "fw1t": np.asarray(inputs["fw1"], dtype=f32).T,
        "fw2t": np.asarray(inputs["fw2"], dtype=f32).T,
        "wlin": (0.5 * w_lin).T,
        "w1b": np.concatenate([w1.T, b1[None, :]], 0),
        "w2b": np.concatenate([w2.T, b2[None, :]], 0),
        "w3b": np.concatenate([w3.T, b3[None, :]], 0),
        "fw3t": np.asarray(inputs["fw3"], dtype=f32).T.reshape(CELL, 1),
    }
    pack0r = np.zeros((128, PACK0R_COLS), f32)
    for name, (c0, ncol) in PACK0R.items():
        arr = np.asarray(parts0r[name], dtype=f32)
        assert arr.shape[1] == ncol and arr.shape[0] <= 128, (name, arr.shape)
        pack0r[: arr.shape[0], c0 : c0 + ncol] = arr
    pack0r = np.ascontiguousarray(pack0r)
    parts0f = {
        "blin": np.tile(b_lin.reshape(1, D1), (128, 1)),
        "mwb": np.tile(
            np.asarray(inputs["modality_weights"], dtype=f32).reshape(1, 2), (128, 1)
        ),
        "fb1": np.asarray(inputs["fb1"], dtype=f32).reshape(CELL, 1),
        "fb2": np.asarray(inputs["fb2"], dtype=f32).reshape(CELL, 1),
        "fb3": np.asarray(inputs["fb3"], dtype=f32).reshape(1, 1),
    }
    pack0f = np.zeros((128, PACK0F_COLS), f32)
    for name, (c0, ncol) in PACK0F.items():
        arr = np.asarray(parts0f[name], dtype=f32)
        pack0f[: arr.shape[0], c0 : c0 + ncol] = arr
    pack0f = np.ascontiguousarray(pack0f)
    parts2 = {
        "whht": whht,
        "mr": np.asarray(inputs["meas_r"], dtype=f32),
        "mi": np.asarray(inputs["meas_i"], dtype=f32),
    }
    pack2 = np.zeros((128, PACK2_COLS), f32)
    for name, (c0, ncol) in PACK2.items():
        arr = np.asarray(parts2[name], dtype=f32)
        pack2[: arr.shape[0], c0 : c0 + ncol] = arr
    pack2 = np.ascontiguousarray(pack2)

    shared = dict(
        lute=lute, lutp=lutp, pack0r=pack0r, pack0f=pack0f, pack1=pack1, pack2=pack2
    )
    in_maps = []
    for c in range(NCORES):
        bs = slice(BL * c, BL * (c + 1))
        # s-major position order: pos = s*BL + b
        wi_c = np.ascontiguousarray(wi_full[bs].T.reshape(NPOS, 1))
        au_c = np.ascontiguousarray(
            au_full[bs].transpose(1, 0, 2).reshape(NPOS, AUDIO)
        )
        m = dict(shared)
        m["wi"] = wi_c
        m["au"] = au_c
        in_maps.append(m)
    return in_maps


def kernel(**inputs):
    from concourse.bass_utils import run_bass_kernel_spmd

    nc = _get_nc()
    in_maps = make_in_maps(inputs)
    res = run_bass_kernel_spmd(nc, in_maps, core_ids=list(range(NCORES)))
    out = np.concatenate(
        [np.asarray(res.results[c]["y"]).reshape(BL, 1) for c in range(NCORES)], axis=0
    ).astype(np.float32)
    return out


# revision 32
# speedup vs baseline: 1.0039x; 1.0039x over previous
"""Trainium2 Bass kernel for nn_LocalMixtureNN (self-contained).

Strategy
--------
Pure data parallel over batch: 8 cores x 4 batches. Within a core the 128
(s, b) positions live on the 128 SBUF partitions (pos = s*BL + b, s-major so
n-gram window shifts are partition shifts by BL*k, realized as matmuls with
constant shift matrices).

v3 notes (92.5us, vs 161us v1 baseline):
- All matmuls are single-pass: bf16 where rounding is tolerable (LSTM
  gates/GX/final MLP/mixing; in-kernel casts), float32r elsewhere. fp32
  2-pass matmuls are never used.
- Both softmaxes use exp(x) = (1+tanh(x/2))/(1-tanh(x/2)) so the Exp table
  is never needed; all Sin ACTs are forced (fake deps) before the first
  Tanh so exactly two ACT table loads occur, both off the critical path.
- LSTM gate pre-activations live in one persistent PSUM bank, zero-filled
  by a single start=True matmul (start=True clears has_written bank-wide,
  so it must happen exactly once per bank); GX matmuls and the per-step
  recurrent-gate matmuls all accumulate with start=False. ACT reads the
  PSUM directly; steady-state step cadence ~1.66us.
- The measurement pipeline runs in two 64-position chunks: chunk 0
  (steps 0..15) is interleaved into LSTM steps 18..30, only chunk 1 forms
  the tail. Chunk c owns partition range [64c, 64c+64) of shared tiles
  (matmul PSUM outputs must start at partition 0; DVE ops may mix
  32-aligned partition bases between operands).
- The real/imag tensor-product combines ride PSUM accumulation of paired
  transposes; rsqrt is a bitcast magic seed + 2 Newton iterations; the
  1/kn^2 and 1/(n0*n1)^2 scales are plain DVE reciprocals applied late.
"""

import numpy as np

try:
    import concourse  # noqa: F401
except ImportError:  # pragma: no cover
    import sys

    sys.path.insert(0, "/opt/trn_rl_repo")

from contextlib import ExitStack

import concourse.bass as bass
import concourse.bacc as bacc
import concourse.tile as tile
import concourse.mybir as mybir

dt = mybir.dt
F32 = dt.float32
R32 = dt.float32r
BF16 = dt.bfloat16
I32 = dt.int32
AF = mybir.ActivationFunctionType
OP = mybir.AluOpType

NCORES = 8
B, S = 32, 32
BL = B // NCORES  # 4 batches per core
NPOS = S * BL  # 128 positions per core, pos = s*BL + b
VOCAB, EMBD, AUDIO, HID = 5000, 300, 74, 128
D1 = 16
D2 = 16
DIM = D1 * D2  # 256
UNITS, CELL = 128, 64
HALF_PI = float(np.pi / 2)
MAGIC = 0x5F3759DF

# pack0r: matmul constants (all float32r)
PACK0R = {}
_c = 0
for _name, _ncol in (
    ("ident", 128), ("s1t", 128), ("s2t", 128), ("fw1t", 64), ("fw2t", 64),
    ("wlin", 16), ("w1b", 16), ("w2b", 16), ("w3b", 16), ("fw3t", 1),
):
    PACK0R[_name] = (_c, _ncol)
    _c += _ncol
PACK0R_COLS = _c  # 577
# pack0f: DVE-side constants (float32)
PACK0F = {}
_c = 0
for _name, _ncol in (
    ("blin", 16), ("mwb", 2), ("fb1", 1), ("fb2", 1), ("fb3", 1),
):
    PACK0F[_name] = (_c, _ncol)
    _c += _ncol
PACK0F_COLS = _c  # 21
# pack2: recurrent weights + measurement kernel
PACK2 = {}
_c = 0
for _name, _ncol in (("whht", 512), ("mr", 256), ("mi", 256)):
    PACK2[_name] = (_c, _ncol)
    _c += _ncol
PACK2_COLS = _c  # 1024


def _outer_bcast(ap, n):
    """AP reading t[p, j] broadcast over a new leading free dim of size n."""
    return bass.AP(tensor=ap.tensor, offset=ap.offset, ap=[ap.ap[0], [0, n], ap.ap[1]])


def build_nc():
    nc = bacc.Bacc("TRN2", target_bir_lowering=False, debug=False)

    # ---------------- DRAM tensors (per-core inputs) ----------------
    wi_d = nc.dram_tensor("wi", [NPOS, 1], I32, kind="ExternalInput")
    au_d = nc.dram_tensor("au", [NPOS, AUDIO], R32, kind="ExternalInput")
    lute_d = nc.dram_tensor("lute", [VOCAB, EMBD], R32, kind="ExternalInput")
    lutp_d = nc.dram_tensor("lutp", [VOCAB, D1 + D2], F32, kind="ExternalInput")
    pack0r_d = nc.dram_tensor("pack0r", [128, PACK0R_COLS], R32, kind="ExternalInput")
    pack0f_d = nc.dram_tensor("pack0f", [128, PACK0F_COLS], F32, kind="ExternalInput")
    pack1_d = nc.dram_tensor("pack1", [128, 1536], R32, kind="ExternalInput")
    pack2_d = nc.dram_tensor("pack2", [128, PACK2_COLS], R32, kind="ExternalInput")
    y_d = nc.dram_tensor("y", [BL, 1], F32, kind="ExternalOutput")

    with tile.TileContext(nc) as tc, ExitStack() as ctx:
        cpool = ctx.enter_context(tc.tile_pool(name="const", bufs=1))
        wpool = ctx.enter_context(tc.tile_pool(name="work", bufs=2))
        npool = ctx.enter_context(tc.tile_pool(name="newton", bufs=2))
        lpool = ctx.enter_context(tc.tile_pool(name="lstm", bufs=3))
        gpall = ctx.enter_context(tc.tile_pool(name="gpall", bufs=1, space="PSUM"))
        ps_t = ctx.enter_context(tc.tile_pool(name="pst", bufs=3, space="PSUM"))
        ps_m = ctx.enter_context(tc.tile_pool(name="psm", bufs=2, space="PSUM"))
        ps_j = ctx.enter_context(tc.tile_pool(name="psj", bufs=1, space="PSUM"))

        # ---------------- bulk loads (criticality order) ----------------
        # wi rides alone at the head of the sync ring so the gathers can
        # start as early as possible; the big weight packs share the scalar
        # ring; audio trails the gathers on the gpsimd ring.
        wi = cpool.tile([NPOS, 1], I32, tag="wi")
        nc.sync.dma_start(wi[:], wi_d[:])
        pack1 = cpool.tile([128, 1536], R32, tag="pack1")
        nc.scalar.dma_start(pack1[:], pack1_d[:])
        gath_p = cpool.tile([NPOS, D1 + D2], F32, tag="gath_p")
        nc.gpsimd.indirect_dma_start(
            out=gath_p[:], out_offset=None, in_=lutp_d[:],
            in_offset=bass.IndirectOffsetOnAxis(ap=wi[:, 0:1], axis=0),
        )
        gath_e = cpool.tile([NPOS, EMBD], R32, tag="gath_e")
        nc.gpsimd.indirect_dma_start(
            out=gath_e[:], out_offset=None, in_=lute_d[:],
            in_offset=bass.IndirectOffsetOnAxis(ap=wi[:, 0:1], axis=0),
        )
        pack0r = cpool.tile([128, PACK0R_COLS], R32, tag="pack0r")
        nc.scalar.dma_start(pack0r[:], pack0r_d[:])
        pack2 = cpool.tile([128, PACK2_COLS], R32, tag="pack2")
        nc.scalar.dma_start(pack2[:], pack2_d[:])
        pack0f = cpool.tile([128, PACK0F_COLS], F32, tag="pack0f")
        nc.sync.dma_start(pack0f[:], pack0f_d[:])
        au = cpool.tile([NPOS, AUDIO], R32, tag="au")
        nc.gpsimd.dma_start(au[:], au_d[:])

        def p0r(name, nrow=128):
            c0, ncol = PACK0R[name]
            return pack0r[0:nrow, c0 : c0 + ncol]

        def p0f(name, nrow=128):
            c0, ncol = PACK0F[name]
            return pack0f[0:nrow, c0 : c0 + ncol]

        def p2(name, nrow=128):
            c0, ncol = PACK2[name]
            return pack2[0:nrow, c0 : c0 + ncol]

        wihta = pack1[:, 0:512]
        wihtb = pack1[:, 512:1024]
        wihtc = pack1[0:45, 1024:1536]
        whht = p2("whht")
        mr = p2("mr")
        mi = p2("mi")
        ident = p0r("ident")
        s1t = p0r("s1t")
        s2t = p0r("s2t")
        fw1t = p0r("fw1t")
        fw2t = p0r("fw2t", 64)
        wlin = p0r("wlin")
        w1b = p0r("w1b", 75)
        w2b = p0r("w2b", 17)
        w3b = p0r("w3b", 17)
        fw3t = p0r("fw3t", 64)
        blin_bc = p0f("blin")
        mwb = p0f("mwb")
        fb1 = p0f("fb1", 64)
        fb2 = p0f("fb2", 64)
        fb3 = p0f("fb3", 1)

        # zero hidden state for step 0 + audio layout tiles (ones rows)
        hz = cpool.tile([HID, BL], BF16, tag="hz")
        nc.gpsimd.memset(hz[:], 0.0)
        auT = cpool.tile([96, NPOS], R32, tag="auT")
        nc.gpsimd.memset(auT[:].bitcast(F32), 1.0)
        a1b = cpool.tile([32, NPOS], R32, tag="a1b")
        nc.gpsimd.memset(a1b[:].bitcast(F32), 1.0)
        a2b = cpool.tile([32, NPOS], R32, tag="a2b")
        nc.gpsimd.memset(a2b[:].bitcast(F32), 1.0)

        def rsqrt_magic(eng, x_ap, tag, iters=2):
            """r ~= rsqrt(x) via bitcast seed + Newton; x_ap is (128,1)."""
            vi = npool.tile([128, 1], I32, tag=f"{tag}_vi")
            eng.tensor_scalar(
                out=vi[:], in0=x_ap.bitcast(I32), scalar1=1, scalar2=None,
                op0=OP.logical_shift_right,
            )
            si = npool.tile([128, 1], I32, tag=f"{tag}_si")
            eng.tensor_scalar(
                out=si[:], in0=vi[:], scalar1=-1, scalar2=MAGIC,
                op0=OP.mult, op1=OP.add,
            )
            hx = npool.tile([128, 1], F32, tag=f"{tag}_hx")
            eng.tensor_scalar(
                out=hx[:], in0=x_ap, scalar1=-0.5, scalar2=None, op0=OP.mult
            )
            cur = si[:].bitcast(F32)
            for i in range(iters):
                z = npool.tile([128, 1], F32, tag=f"{tag}_z")
                eng.scalar_tensor_tensor(
                    out=z[:], in0=cur, scalar=hx[:, 0:1], in1=cur,
                    op0=OP.mult, op1=OP.mult,
                )
                nxt = npool.tile([128, 1], F32, tag=f"{tag}_r")
                eng.scalar_tensor_tensor(
                    out=nxt[:], in0=z[:], scalar=1.5, in1=cur,
                    op0=OP.add, op1=OP.mult,
                )
                cur = nxt[:]
            return cur

        # ---------------- trig ----------------
        PI = float(np.pi)
        TWO_PI = float(2 * np.pi)
        ph0g = gath_p[:, 0:D1]
        ph1g = gath_p[:, D1 : D1 + D2]

        def wrap3(src_ap, width, tag):
            cur = src_ap
            for p in range(3):
                t = wpool.tile([128, width], F32, tag=f"{tag}_w")
                nc.vector.add_range_wrap(
                    out=t[:], in_=cur, shift=0.0, bound=PI, period=TWO_PI
                )
                cur = t[:]
            return cur

        ph0w = wrap3(ph0g, D1, "wr0")
        ph1w = wrap3(ph1g, D2, "wr1")
        ph0c = wpool.tile([128, D1], F32, tag="ca0")
        nc.vector.add_range_wrap(
            out=ph0c[:], in_=ph0w, shift=HALF_PI, bound=PI, period=TWO_PI
        )
        ph1c = wpool.tile([128, D2], F32, tag="ca1")
        nc.vector.add_range_wrap(
            out=ph1c[:], in_=ph1w, shift=HALF_PI, bound=PI, period=TWO_PI
        )
        cos0 = cpool.tile([NPOS, D1], F32, tag="cos0")
        nc.scalar.activation(out=cos0[:], in_=ph0c[:], func=AF.Sin)
        sin0 = cpool.tile([NPOS, D1], F32, tag="sin0")
        nc.scalar.activation(out=sin0[:], in_=ph0w, func=AF.Sin)
        cos1 = cpool.tile([NPOS, D2], F32, tag="cos1")
        nc.scalar.activation(out=cos1[:], in_=ph1c[:], func=AF.Sin)
        sin1 = cpool.tile([NPOS, D2], F32, tag="sin1")
        nc.scalar.activation(out=sin1[:], in_=ph1w, func=AF.Sin)

        # modality softmax over 2: mw0 = sigmoid(a-b) via tanh.
        # dm is given artificial deps on all four Sin outputs so every Sin
        # is scheduled before the first Tanh: exactly two ACT table loads
        # (trig at entry, then the tanh set), both off the critical path.
        dm = wpool.tile([128, 1], F32, tag="dm")
        nc.vector.tensor_tensor(
            out=dm[:], in0=mwb[:, 0:1], in1=mwb[:, 1:2], op=OP.subtract
        )
        j1 = wpool.tile([128, 1], F32, tag="j1")
        nc.vector.scalar_tensor_tensor(
            out=j1[:], in0=cos0[:, 0:1], scalar=0.0, in1=sin0[:, 0:1],
            op0=OP.mult, op1=OP.mult,
        )
        j2 = wpool.tile([128, 1], F32, tag="j2")
        nc.vector.scalar_tensor_tensor(
            out=j2[:], in0=cos1[:, 0:1], scalar=0.0, in1=sin1[:, 0:1],
            op0=OP.mult, op1=OP.mult,
        )
        jj = wpool.tile([128, 1], F32, tag="jj")
        nc.vector.tensor_tensor(out=jj[:], in0=j1[:], in1=j2[:], op=OP.add)
        dm2 = wpool.tile([128, 1], F32, tag="dm2")
        nc.vector.scalar_tensor_tensor(
            out=dm2[:], in0=jj[:], scalar=1.0, in1=dm[:], op0=OP.mult, op1=OP.add
        )
        tmw = wpool.tile([128, 1], F32, tag="tmw")
        nc.scalar.activation(out=tmw[:], in_=dm2[:], func=AF.Tanh, scale=0.5)
        mw0 = cpool.tile([128, 1], F32, tag="mw0")
        nc.vector.tensor_scalar(
            out=mw0[:], in0=tmw[:], scalar1=0.5, scalar2=0.5, op0=OP.mult, op1=OP.add
        )
        mw1 = cpool.tile([128, 1], F32, tag="mw1")
        nc.vector.tensor_scalar(
            out=mw1[:], in0=tmw[:], scalar1=-0.5, scalar2=0.5, op0=OP.mult, op1=OP.add
        )

        # ---------------- embedding transpose ----------------
        embT0 = cpool.tile([128, NPOS], BF16, tag="embT0")
        pst = ps_t.tile([128, 128], F32, tag="pst")
        nc.tensor.transpose(pst[:].bitcast(R32), gath_e[:, 0:128], ident)
        nc.vector.tensor_copy(out=embT0[:], in_=pst[:])
        embT1 = cpool.tile([128, NPOS], BF16, tag="embT1")
        pst = ps_t.tile([128, 128], F32, tag="pst")
        nc.tensor.transpose(pst[:].bitcast(R32), gath_e[:, 128:256], ident)
        nc.vector.tensor_copy(out=embT1[:], in_=pst[:])
        embT2 = cpool.tile([64, NPOS], BF16, tag="embT2")
        nc.gpsimd.memset(embT2[:], 1.0)
        pst44 = ps_t.tile([44, 128], F32, tag="pst")
        nc.tensor.transpose(pst44[:].bitcast(R32), gath_e[:, 256:300], ident)
        nc.vector.tensor_copy(out=embT2[0:44, :], in_=pst44[:])

        ident_b = cpool.tile([128, 128], BF16, tag="ident_b")
        nc.vector.tensor_copy(out=ident_b[:], in_=ident)
        identn_b = cpool.tile([128, 128], BF16, tag="identn_b")
        nc.vector.tensor_scalar(
            out=identn_b[:], in0=ident, scalar1=-1.0, scalar2=None, op0=OP.mult
        )
        s1t_b = cpool.tile([128, 128], BF16, tag="s1t_b")
        nc.vector.tensor_copy(out=s1t_b[:], in_=s1t)
        s2t_b = cpool.tile([128, 128], BF16, tag="s2t_b")
        nc.vector.tensor_copy(out=s2t_b[:], in_=s2t)
        fw1t_b = cpool.tile([128, 64], BF16, tag="fw1t_b")
        nc.vector.tensor_copy(out=fw1t_b[:], in_=fw1t)
        fw2t_b = cpool.tile([64, 64], BF16, tag="fw2t_b")
        nc.vector.tensor_copy(out=fw2t_b[:], in_=fw2t)
        fw3t_b = cpool.tile([64, 1], BF16, tag="fw3t_b")
        nc.vector.tensor_copy(out=fw3t_b[:], in_=fw3t)
        wihta_b = cpool.tile([128, 512], BF16, tag="wihta_b")
        nc.vector.tensor_copy(out=wihta_b[:], in_=wihta)
        wihtb_b = cpool.tile([128, 512], BF16, tag="wihtb_b")
        nc.vector.tensor_copy(out=wihtb_b[:], in_=wihtb)
        wihtc_b = cpool.tile([45, 512], BF16, tag="wihtc_b")
        nc.vector.tensor_copy(out=wihtc_b[:], in_=wihtc)
        # ---------------- GX accumulation into persistent PSUM ----------------
        # gp_all[hid, s, gate, b] holds the gate pre-activations; GX matmuls
        # accumulate the input part, each LSTM step adds the recurrent part.
        gp_all = gpall.tile([HID, 4, S, BL], F32, tag="gp_all")
        GX_BLOCKS = ((0, 8), (8, 32))
        # one start=True matmul zero-fills the whole bank: a start=True
        # clears has_written bank-wide, so it must happen exactly once here
        # and every later write into this bank accumulates (start=False).
        zmov = cpool.tile([128, 4 * S * BL], R32, tag="zmov")
        nc.gpsimd.memset(zmov[:].bitcast(F32), 0.0)
        nc.tensor.matmul(
            gp_all[:, :, :, :], ident, zmov[:],
            start=True, stop=False, skip_group_check=True,
        )

        def emit_gx(blk, g):
            s0, s1 = GX_BLOCKS[blk]
            p0, p1 = s0 * BL, s1 * BL
            gsl = slice(HID * g, HID * (g + 1))
            out_ap = gp_all[:, g, s0:s1, :]
            nc.tensor.matmul(
                out_ap, wihta_b[:, gsl], embT0[:, p0:p1],
                start=False, stop=False, skip_group_check=True,
            )
            nc.tensor.matmul(
                out_ap, wihtb_b[:, gsl], embT1[:, p0:p1],
                start=False, stop=False, skip_group_check=True,
            )
            nc.tensor.matmul(
                out_ap, wihtc_b[:, gsl], embT2[0:45, p0:p1],
                start=False, stop=False, skip_group_check=True,
            )

        for g in range(4):
            emit_gx(0, g)

        # ---------------- LSTM step emission ----------------
        H2 = cpool.tile([HID, NPOS], BF16, tag="H2")
        whht_b = cpool.tile([HID, 512], BF16, tag="whht_b")
        nc.vector.tensor_copy(out=whht_b[:], in_=whht)
        wlin_b = cpool.tile([128, D1], BF16, tag="wlin_b")
        nc.vector.tensor_copy(out=wlin_b[:], in_=wlin)
        state = {"c2": None}
        # scratch bank for PE-warming matmuls (keeps the HAM clock gate at
        # 2.4GHz during the LSTM; results are never read)
        junk_ps = ps_j.tile([128, 448], F32, tag="junk_ps")

        def pe_warm(n=3):
            for _ in range(n):
                nc.tensor.matmul(
                    junk_ps[:, 0:448], ident, zmov[:, 0:448],
                    start=True, stop=True, skip_group_check=True,
                )

        def emit_step(s):
            hprev = hz[:] if s == 0 else H2[:, (s - 1) * BL : s * BL]
            for g in range(3):
                nc.tensor.matmul(
                    gp_all[:, g, s, :], whht_b[:, HID * g : HID * (g + 1)], hprev,
                    start=False, stop=True, skip_group_check=True,
                )
            g3 = lpool.tile([128, 3, BL], F32, tag="g3")
            nc.scalar.activation(out=g3[:], in_=gp_all[:, 0:3, s, :], func=AF.Tanh)
            nc.tensor.matmul(
                gp_all[:, 3, s, :], whht_b[:, HID * 3 : HID * 4], hprev,
                start=False, stop=True, skip_group_check=True,
            )
            tot = lpool.tile([128, BL], F32, tag="tot")
            nc.scalar.activation(out=tot[:], in_=gp_all[:, 3, s, :], func=AF.Tanh)
            ti, tf, tg = g3[:, 0, :], g3[:, 1, :], g3[:, 2, :]
            q2 = lpool.tile([128, BL], F32, tag="q2")
            nc.vector.scalar_tensor_tensor(
                out=q2[:], in0=ti, scalar=1.0, in1=tg, op0=OP.add, op1=OP.mult
            )
            if s == 0:
                c2 = q2
            else:
                q1 = lpool.tile([128, BL], F32, tag="q1")
                nc.vector.scalar_tensor_tensor(
                    out=q1[:], in0=tf, scalar=1.0, in1=state["c2"][:],
                    op0=OP.add, op1=OP.mult,
                )
                c2 = lpool.tile([128, BL], F32, tag="c2")
                nc.vector.scalar_tensor_tensor(
                    out=c2[:], in0=q1[:], scalar=0.5, in1=q2[:],
                    op0=OP.mult, op1=OP.add,
                )
            th = lpool.tile([128, BL], F32, tag="th")
            nc.scalar.activation(out=th[:], in_=c2[:], func=AF.Tanh, scale=0.5)
            nc.vector.scalar_tensor_tensor(
                out=H2[:, s * BL : (s + 1) * BL], in0=tot[:], scalar=1.0, in1=th[:],
                op0=OP.add, op1=OP.mult,
            )
            state["c2"] = c2

        # ---------------- fillers (emitted between LSTM steps) ----------------
        post = {}

        def audio_mm1():
            psau = ps_t.tile([AUDIO, NPOS], F32, tag="pst")
            nc.tensor.transpose(psau[:].bitcast(R32), au[:], ident)
            nc.vector.tensor_copy(out=auT[0:AUDIO, :], in_=psau[:])

        def audio_mm2():
            psm = ps_t.tile([D2, NPOS], F32, tag="pst")
            nc.tensor.matmul(psm[:], w1b, auT[0 : AUDIO + 1, :], start=True, stop=True)
            nc.vector.tensor_scalar(
                out=a1b[0:D2, :], in0=psm[:], scalar1=0.0, scalar2=None, op0=OP.max
            )

        def audio_mm3():
            psm = ps_t.tile([D2, NPOS], F32, tag="pst")
            nc.tensor.matmul(psm[:], w2b, a1b[0 : D2 + 1, :], start=True, stop=True)
            nc.vector.tensor_scalar(
                out=a2b[0:D2, :], in0=psm[:], scalar1=0.0, scalar2=None, op0=OP.max
            )

        amp1T = cpool.tile([D2, NPOS], R32, tag="amp1T")
        amp1 = cpool.tile([NPOS, D2], F32, tag="amp1")

        def audio_mm4():
            psm = ps_t.tile([D2, NPOS], F32, tag="pst")
            nc.tensor.matmul(psm[:], w3b, a2b[0 : D2 + 1, :], start=True, stop=True)
            nc.vector.tensor_scalar(
                out=amp1T[:], in0=psm[:], scalar1=0.0, scalar2=None, op0=OP.max
            )

        def audio_tr():
            ps16 = ps_t.tile([128, D2], F32, tag="pst")
            nc.tensor.transpose(ps16[:].bitcast(R32), amp1T[:], ident[0:D2, 0:D2])
            nc.vector.tensor_copy(out=amp1[:], in_=ps16[:])

        s1x = cpool.tile([128, 1], F32, tag="s1x")
        wpart = cpool.tile([128, 1], F32, tag="wpart")
        n1chain = {}

        def n1_a():
            junk16 = wpool.tile([128, D2], F32, tag="junk16")
            nc.vector.scalar_tensor_tensor(
                out=junk16[:], in0=amp1[:], scalar=1.0, in1=amp1[:],
                op0=OP.mult, op1=OP.mult, accum_out=s1x[:, 0:1],
            )

        def n1_b():
            n1chain["r"] = rsqrt_magic(nc.vector, s1x[:], "n1")

        def n1_c():
            n1 = wpool.tile([128, 1], F32, tag="n1v")
            nc.vector.tensor_scalar(
                out=n1[:], in0=s1x[:], scalar1=n1chain["r"][:, 0:1], scalar2=None,
                op0=OP.mult,
            )
            nc.vector.tensor_scalar(
                out=wpart[:], in0=n1[:], scalar1=mw1[:, 0:1], scalar2=None, op0=OP.mult
            )

        rkn2 = cpool.tile([128, 1], F32, tag="rkn2")

        def kn_chain():
            junk256 = wpool.tile([128, DIM], F32, tag="junk256")
            skr = wpool.tile([128, 1], F32, tag="skr")
            nc.vector.scalar_tensor_tensor(
                out=junk256[:], in0=mr, scalar=1.0, in1=mr,
                op0=OP.mult, op1=OP.mult, accum_out=skr[:, 0:1],
            )
            ski = wpool.tile([128, 1], F32, tag="ski")
            nc.vector.scalar_tensor_tensor(
                out=junk256[:], in0=mi, scalar=1.0, in1=mi,
                op0=OP.mult, op1=OP.mult, accum_out=ski[:, 0:1],
            )
            kx = wpool.tile([128, 1], F32, tag="kx")
            nc.gpsimd.tensor_tensor(out=kx[:], in0=skr[:], in1=ski[:], op=OP.add)
            nc.vector.reciprocal(out=rkn2[:], in_=kx[:])

        vrviT = []

        def vrvi(k):
            t = cpool.tile([128, 2 * UNITS], BF16, tag=f"vrviT{k}")
            for half, srct in ((0, mr), (1, mi)):
                pstv = ps_t.tile([128, 128], F32, tag="pst")
                nc.tensor.transpose(
                    pstv[:].bitcast(R32), srct[:, 128 * k : 128 * (k + 1)], ident
                )
                nc.scalar.copy(out=t[:, 128 * half : 128 * (half + 1)], in_=pstv[:])
            vrviT.append(t)

        r1v = cpool.tile([NPOS, D2], F32, tag="r1v")
        i1v = cpool.tile([NPOS, D2], F32, tag="i1v")
        uv = cpool.tile([NPOS, D2], F32, tag="uv")
        vv = cpool.tile([NPOS, D2], F32, tag="vv")

        def branch1():
            nc.gpsimd.tensor_tensor(out=r1v[:], in0=amp1[:], in1=cos1[:], op=OP.mult)
            nc.gpsimd.tensor_tensor(out=i1v[:], in0=amp1[:], in1=sin1[:], op=OP.mult)
            nc.gpsimd.tensor_tensor(out=uv[:], in0=r1v[:], in1=i1v[:], op=OP.subtract)
            nc.gpsimd.tensor_tensor(out=vv[:], in0=r1v[:], in1=i1v[:], op=OP.add)

        # ---------------- post-phase chunks (64 positions each) ----------
        # Chunk 0 (positions 0..63, steps 0..15) runs as fillers inside the
        # LSTM; chunk 1 forms the tail. Shared tiles, chunk c owns the
        # 32-aligned partition range [64c, 64c+64).
        amp0 = cpool.tile([NPOS, D1], F32, tag="amp0")
        s0x = cpool.tile([128, 1], F32, tag="s0x")
        csqt = cpool.tile([128, 1], F32, tag="csqt")
        r0v = cpool.tile([NPOS, D1], F32, tag="r0v")
        i0v = cpool.tile([NPOS, D1], F32, tag="i0v")
        tmpA = cpool.tile([NPOS, DIM], BF16, tag="tmpA")
        tmpB = cpool.tile([NPOS, DIM], BF16, tag="tmpB")
        tmpC = cpool.tile([NPOS, DIM], BF16, tag="tmpC")
        tmpD = cpool.tile([NPOS, DIM], BF16, tag="tmpD")
        rimT = cpool.tile([128, 4, NPOS], BF16, tag="rimT")
        rt_sb = cpool.tile([NPOS, 2 * UNITS], F32, tag="rt_sb")
        u1 = cpool.tile([NPOS, UNITS], F32, tag="u1")
        u2 = cpool.tile([NPOS, UNITS], F32, tag="u2")
        sq1 = cpool.tile([NPOS, UNITS], F32, tag="sq1")
        sq2 = cpool.tile([NPOS, UNITS], F32, tag="sq2")
        msu = cpool.tile([NPOS, UNITS], F32, tag="msu")
        msr = cpool.tile([NPOS, UNITS], BF16, tag="msr")
        pq_ps = {}
        rt_ps = {}

        def PSL(c):
            return slice(64 * c, 64 * (c + 1))

        def pc_amp0(c):
            P = PSL(c)
            pa = ps_t.tile([64, D1], F32, tag="pst")
            nc.tensor.matmul(
                pa[:, :], H2[:, P], wlin_b[:], start=True, stop=True,
                skip_group_check=True,
            )
            nc.vector.tensor_tensor(
                out=amp0[P, :], in0=pa[:, :], in1=blin_bc[P, :], op=OP.add
            )

        def pc_norm(c):
            P = PSL(c)
            junk16b = wpool.tile([128, D1], F32, tag="junk16b")
            nc.vector.scalar_tensor_tensor(
                out=junk16b[P, :], in0=amp0[P, :], scalar=1.0, in1=amp0[P, :],
                op0=OP.mult, op1=OP.mult, accum_out=s0x[P, 0:1],
            )
            prod = wpool.tile([128, 1], F32, tag="prod")
            nc.vector.tensor_scalar(
                out=prod[P, :], in0=s0x[P, :], scalar1=s1x[P, 0:1], scalar2=1e-30,
                op0=OP.mult, op1=OP.add,
            )
            nc.vector.reciprocal(out=csqt[P, :], in_=prod[P, :])
            nc.vector.tensor_tensor(
                out=r0v[P, :], in0=amp0[P, :], in1=cos0[P, :], op=OP.mult
            )
            nc.vector.tensor_tensor(
                out=i0v[P, :], in0=amp0[P, :], in1=sin0[P, :], op=OP.mult
            )

        def pc_outer_re(c):
            P = PSL(c)
            eng = nc.gpsimd if c == 0 else nc.vector
            eng.tensor_tensor(
                out=tmpA[P, :].rearrange("p (i j) -> p i j", j=D2),
                in0=r0v[P, :].to_broadcast([64, D1, D2]),
                in1=_outer_bcast(uv[P, :], D1), op=OP.mult,
            )
            eng.tensor_tensor(
                out=tmpB[P, :].rearrange("p (i j) -> p i j", j=D2),
                in0=i0v[P, :].to_broadcast([64, D1, D2]),
                in1=_outer_bcast(vv[P, :], D1), op=OP.mult,
            )

        def pc_outer_im(c):
            P = PSL(c)
            nc.gpsimd.tensor_tensor(
                out=tmpC[P, :].rearrange("p (i j) -> p i j", j=D2),
                in0=r0v[P, :].to_broadcast([64, D1, D2]),
                in1=_outer_bcast(vv[P, :], D1), op=OP.mult,
            )
            nc.gpsimd.tensor_tensor(
                out=tmpD[P, :].rearrange("p (i j) -> p i j", j=D2),
                in0=i0v[P, :].to_broadcast([64, D1, D2]),
                in1=_outer_bcast(uv[P, :], D1), op=OP.mult,
            )

        def pc_trans(c, qs):
            # rimT[:, q, P] = transpose(tmpA_q) + transpose(tmpB_q): the real/
            # imag combines ride the PSUM accumulation of paired transposes.
            P = PSL(c)
            for q in qs:
                t1, t2 = (tmpA, tmpB) if q < 2 else (tmpC, tmpD)
                id2 = identn_b if q < 2 else ident_b
                qq = 128 * (q % 2)
                pstq = ps_t.tile([128, 64], F32, tag="pst")
                nc.tensor.matmul(
                    pstq[:], t1[P, qq : qq + 128], ident_b[P, P],
                    start=True, stop=False, skip_group_check=True,
                )
                nc.tensor.matmul(
                    pstq[:], t2[P, qq : qq + 128], id2[P, P],
                    start=False, stop=True, skip_group_check=True,
                )
                if c == 0:
                    nc.vector.tensor_copy(out=rimT[:, q, P], in_=pstq[:])
                else:
                    nc.scalar.copy(out=rimT[:, q, P], in_=pstq[:])

        def pc_pqrt(c, which):
            P = PSL(c)
            t = ps_m.tile([64, 2 * UNITS], F32, tag="psm")
            q0 = 0 if which == 0 else 2
            nc.tensor.matmul(
                t[:, :], rimT[:, q0, P], vrviT[0][:],
                start=True, stop=False, skip_group_check=True,
            )
            nc.tensor.matmul(
                t[:, :], rimT[:, q0 + 1, P], vrviT[1][:],
                start=False, stop=True, skip_group_check=True,
            )
            (pq_ps if which == 0 else rt_ps)[c] = t

        def pc_u(c):
            P = PSL(c)
            nc.vector.tensor_copy(out=rt_sb[P, :], in_=rt_ps[c][:, :])
            nc.vector.tensor_tensor(
                out=u1[P, :], in0=pq_ps[c][:, 0:UNITS],
                in1=rt_sb[P, UNITS : 2 * UNITS], op=OP.add,
            )
            nc.vector.tensor_tensor(
                out=u2[P, :], in0=rt_sb[P, 0:UNITS],
                in1=pq_ps[c][:, UNITS : 2 * UNITS], op=OP.subtract,
            )

        def pc_msr(c):
            P = PSL(c)
            nc.scalar.activation(out=sq1[P, :], in_=u1[P, :], func=AF.Square)
            nc.scalar.activation(out=sq2[P, :], in_=u2[P, :], func=AF.Square)
            nc.vector.tensor_tensor(
                out=msu[P, :], in0=sq1[P, :], in1=sq2[P, :], op=OP.add
            )
            nc.vector.tensor_scalar(
                out=msr[P, :], in0=msu[P, :], scalar1=csqt[P, 0:1], scalar2=None,
                op0=OP.mult,
            )

        # GX block 1 split into per-gate chunks to interleave with early steps
        post[1] = [lambda: emit_gx(1, 0), audio_mm1]
        post[2] = [lambda: emit_gx(1, 1), audio_mm2]
        post[3] = [lambda: emit_gx(1, 2), audio_mm3]
        post[4] = [lambda: emit_gx(1, 3), audio_mm4]
        post[5] = [audio_tr, lambda: vrvi(0)]
        post[6] = [lambda: vrvi(1), n1_a]
        post[7] = [n1_b]
        post[8] = [n1_c, kn_chain]
        post[9] = [branch1]
        post[18] = [lambda: pc_amp0(0)]
        post[19] = [lambda: pc_norm(0)]
        post[20] = [lambda: pc_outer_re(0)]
        post[21] = [lambda: pc_outer_im(0)]
        post[22] = [lambda: pc_trans(0, (0,))]
        post[23] = [lambda: pc_trans(0, (1,))]
        post[24] = [lambda: pc_trans(0, (2,))]
        post[25] = [lambda: pc_trans(0, (3,))]
        post[26] = [lambda: pc_pqrt(0, 0)]
        post[27] = [lambda: pc_pqrt(0, 1)]
        post[29] = [lambda: pc_u(0)]
        post[30] = [lambda: pc_msr(0)]

        for s in range(S):
            emit_step(s)
            for fn in post.get(s, ()):
                fn()

        # ---------------- tail: chunk 1 + weight path ----------------
        pc_amp0(1)
        pc_norm(1)
        pc_outer_re(1)
        pc_outer_im(1)
        pc_trans(1, (0, 1, 2, 3))
        pc_pqrt(1, 0)
        pc_pqrt(1, 1)
        # weight / windowed-softmax path (needs s0x of both chunks); its DVE
        # work fills gaps while the PE does chunk-1 transposes + PQRT.
        r0n = rsqrt_magic(nc.vector, s0x[:], "n0")
        n0 = wpool.tile([128, 1], F32, tag="n0v")
        nc.vector.tensor_scalar(
            out=n0[:], in0=s0x[:], scalar1=r0n[:, 0:1], scalar2=None, op0=OP.mult
        )
        weight = wpool.tile([128, 2], BF16, tag="weight")
        nc.vector.scalar_tensor_tensor(
            out=weight[:], in0=n0[:].to_broadcast([128, 2]),
            scalar=mw0[:, 0:1], in1=wpart[:].to_broadcast([128, 2]),
            op0=OP.mult, op1=OP.add,
        )
        # windowed weights V = [w, shift4(w), shift8(w)] then softmax (tanh-exp)
        vp = ps_t.tile([128, 4], F32, tag="pst")
        nc.tensor.matmul(vp[:, 0:2], s1t_b[:], weight[:], start=True, stop=True)
        nc.tensor.matmul(vp[:, 2:4], s2t_b[:], weight[:], start=True, stop=True)
        v3 = wpool.tile([128, 3], F32, tag="v3")
        nc.gpsimd.tensor_copy(out=v3[:, 0:1], in_=weight[:, 0:1])
        nc.vector.tensor_copy(out=v3[:, 1:2], in_=vp[:, 0:1])
        nc.vector.tensor_copy(out=v3[:, 2:3], in_=vp[:, 2:3])
        vmax = wpool.tile([128, 1], F32, tag="vmax")
        nc.vector.tensor_reduce(
            out=vmax[:], in_=v3[:], axis=mybir.AxisListType.X, op=OP.max
        )
        v3s = wpool.tile([128, 3], F32, tag="v3s")
        nc.vector.tensor_scalar(
            out=v3s[:], in0=v3[:], scalar1=vmax[:, 0:1], scalar2=None, op0=OP.subtract
        )
        t3 = wpool.tile([128, 3], F32, tag="t3")
        nc.scalar.activation(out=t3[:], in_=v3s[:], func=AF.Tanh, scale=0.5)
        den3 = wpool.tile([128, 3], F32, tag="den3")
        nc.vector.tensor_scalar(
            out=den3[:], in0=t3[:], scalar1=-1.0, scalar2=1.0, op0=OP.mult, op1=OP.add
        )
        rec3 = wpool.tile([128, 3], F32, tag="rec3")
        nc.vector.reciprocal(out=rec3[:], in_=den3[:])
        e3 = wpool.tile([128, 3], F32, tag="e3")
        nc.vector.scalar_tensor_tensor(
            out=e3[:], in0=t3[:], scalar=1.0, in1=rec3[:], op0=OP.add, op1=OP.mult
        )
        esum = wpool.tile([128, 1], F32, tag="esum")
        nc.vector.tensor_reduce(
            out=esum[:], in_=e3[:], axis=mybir.AxisListType.X, op=OP.add
        )
        res = wpool.tile([128, 1], F32, tag="res")
        nc.vector.reciprocal(out=res[:], in_=esum[:])
        ww = wpool.tile([128, 3], F32, tag="ww")
        nc.vector.tensor_scalar(
            out=ww[:], in0=e3[:], scalar1=res[:, 0:1], scalar2=None, op0=OP.mult
        )
        pc_u(1)
        pc_msr(1)

        # n-gram mixing: m3 = ww0*M + ww1*shift4(M) + ww2*shift8(M)
        ps_ms = ps_m.tile([NPOS, 2 * UNITS], F32, tag="psm")
        nc.tensor.matmul(ps_ms[:, 0:UNITS], s1t_b[:], msr[:], start=True, stop=True)
        nc.tensor.matmul(ps_ms[:, UNITS : 2 * UNITS], s2t_b[:], msr[:], start=True, stop=True)
        t1m = wpool.tile([NPOS, UNITS], F32, tag="t1m")
        nc.vector.tensor_scalar(
            out=t1m[:], in0=ps_ms[:, 0:UNITS], scalar1=ww[:, 1:2], scalar2=None,
            op0=OP.mult,
        )
        m3a = wpool.tile([NPOS, UNITS], F32, tag="m3a")
        nc.vector.scalar_tensor_tensor(
            out=m3a[:], in0=ps_ms[:, UNITS : 2 * UNITS], scalar=ww[:, 2:3], in1=t1m[:],
            op0=OP.mult, op1=OP.add,
        )
        m3 = wpool.tile([NPOS, UNITS], F32, tag="m3")
        nc.vector.scalar_tensor_tensor(
            out=m3[:], in0=msr[:], scalar=ww[:, 0:1], in1=m3a[:],
            op0=OP.mult, op1=OP.add,
        )
        mmx = wpool.tile([NPOS, UNITS], R32, tag="mmx")
        nc.vector.tensor_tensor(out=mmx[:], in0=msr[:], in1=m3[:], op=OP.max)

        # max over positions per batch: transpose then reduce over s
        ps_mt = ps_t.tile([UNITS, NPOS], F32, tag="pst")
        nc.tensor.transpose(ps_mt[:].bitcast(R32), mmx[:], ident)
        featU = wpool.tile([UNITS, BL], F32, tag="featU")
        nc.vector.tensor_reduce(
            out=featU[:], in_=ps_mt[:].rearrange("p (s b) -> p b s", b=BL),
            axis=mybir.AxisListType.X, op=OP.max,
        )
        featT = wpool.tile([UNITS, BL], BF16, tag="featT")
        nc.vector.tensor_scalar(
            out=featT[:], in0=featU[:], scalar1=rkn2[:, 0:1], scalar2=None,
            op0=OP.mult,
        )

        # ---------------- final MLP ----------------
        y1p = ps_t.tile([CELL, BL], F32, tag="pst")
        nc.tensor.matmul(y1p[:], fw1t_b[:], featT[:], start=True, stop=True)
        y1 = wpool.tile([CELL, BL], BF16, tag="y1")
        nc.vector.tensor_scalar(
            out=y1[:], in0=y1p[:], scalar1=fb1[:, 0:1], scalar2=0.0,
            op0=OP.add, op1=OP.max,
        )
        y2p = ps_t.tile([CELL, BL], F32, tag="pst")
        nc.tensor.matmul(y2p[:], fw2t_b[:], y1[:], start=True, stop=True)
        y2 = wpool.tile([CELL, BL], BF16, tag="y2")
        nc.vector.tensor_scalar(
            out=y2[:], in0=y2p[:], scalar1=fb2[:, 0:1], scalar2=0.0,
            op0=OP.add, op1=OP.max,
        )
        y3p = ps_t.tile([1, BL], F32, tag="pst")
        nc.tensor.matmul(y3p[:], fw3t_b[:], y2[:], start=True, stop=True)
        ysb = wpool.tile([1, BL], F32, tag="ysb")
        nc.vector.tensor_scalar(
            out=ysb[:], in0=y3p[:], scalar1=fb3[0:1, 0:1], scalar2=None, op0=OP.add
        )
        nc.sync.dma_start(y_d[:], ysb[:])

    nc.compile()
    return nc


_NC = None


def _get_nc():
    global _NC
    if _NC is None:
        _NC = build_nc()
    return _NC


def _padrows(a, rows):
    out = np.zeros((rows, a.shape[1]), np.float32)
    out[: a.shape[0]] = a
    return out


def make_in_maps(inputs):
    """Host-side layout prep (value-preserving transforms only)."""
    f32 = np.float32
    wi_full = np.asarray(inputs["word_indexes"]).astype(np.int32)  # (B, S)
    au_full = np.asarray(inputs["audio"], dtype=f32)  # (B, S, A)
    lute = np.ascontiguousarray(np.asarray(inputs["lookup_table"], dtype=f32))
    lutp = np.ascontiguousarray(
        np.concatenate(
            [
                np.asarray(inputs["phase_tab0"], dtype=f32),
                np.asarray(inputs["phase_tab1"], dtype=f32),
            ],
            axis=1,
        )
    )
    w_ih = np.asarray(inputs["w_ih"], dtype=f32)
    w_hh = np.asarray(inputs["w_hh"], dtype=f32)
    b_ih = np.asarray(inputs["b_ih"], dtype=f32)
    b_hh = np.asarray(inputs["b_hh"], dtype=f32)
    w_lin = np.asarray(inputs["w_lin"], dtype=f32)
    b_lin = np.asarray(inputs["b_lin"], dtype=f32)

    # gate order [i, f, g, o]; sigma->tanh fold (x0.5 on i,f,o rows);
    # extra x0.5 on all w_hh entries for the doubled hidden state H2=2h.
    gsc = np.concatenate(
        [np.full(256, 0.5, f32), np.ones(128, f32), np.full(128, 0.5, f32)]
    )
    w_ih_p = w_ih * gsc[:, None]
    b_p = (b_ih + b_hh) * gsc
    wihT = np.concatenate([w_ih_p.T, b_p[None, :]], axis=0).astype(f32)  # (301,512)
    whht = (w_hh * gsc[:, None] * 0.5).T.astype(f32)  # (128, 512)

    pack1 = np.concatenate(
        [wihT[0:128], wihT[128:256], _padrows(wihT[256:301], 128)], axis=1
    )
    pack1 = np.ascontiguousarray(pack1, dtype=f32)

    w1 = np.asarray(inputs["w1"], dtype=f32)
    w2 = np.asarray(inputs["w2"], dtype=f32)
    w3 = np.asarray(inputs["w3"], dtype=f32)
    b1 = np.asarray(inputs["b1"], dtype=f32)
    b2 = np.asarray(inputs["b2"], dtype=f32)
    b3 = np.asarray(inputs["b3"], dtype=f32)
    parts0r = {
        "ident": np.eye(128, dtype=f32),
        "s1t": np.eye(128, k=4, dtype=f32).T,
        "s2t": np.eye(128, k=8, dtype=f32).T,
        "fw1t": np.asarray(inputs["fw1"], dtype=f32).T,
        "fw2t": np.asarray(inputs["fw2"], dtype=f32).T,
        "wlin": (0.5 * w_lin).T,
        "w1b": np.concatenate([w1.T, b1[None, :]], 0),
        "w2b": np.concatenate([w2.T, b2[None, :]], 0),
        "w3b": np.concatenate([w3.T, b3[None, :]], 0),
        "fw3t": np.asarray(inputs["fw3"], dtype=f32).T.reshape(CELL, 1),
    }
    pack0r = np.zeros((128, PACK0R_COLS), f32)
    for name, (c0, ncol) in PACK0R.items():
        arr = np.asarray(parts0r[name], dtype=f32)
        assert arr.shape[1] == ncol and arr.shape[0] <= 128, (name, arr.shape)
        pack0r[: arr.shape[0], c0 : c0 + ncol] = arr
    pack0r = np.ascontiguousarray(pack0r)
    parts0f = {
        "blin": np.tile(b_lin.reshape(1, D1), (128, 1)),
        "mwb": np.tile(
            np.asarray(inputs["modality_weights"], dtype=f32).reshape(1, 2), (128, 1)
        ),
        "fb1": np.asarray(inputs["fb1"], dtype=f32).reshape(CELL, 1),
        "fb2": np.asarray(inputs["fb2"], dtype=f32).reshape(CELL, 1),
        "fb3": np.asarray(inputs["fb3"], dtype=f32).reshape(1, 1),
    }
    pack0f = np.zeros((128, PACK0F_COLS), f32)
    for name, (c0, ncol) in PACK0F.items():
        arr = np.asarray(parts0f[name], dtype=f32)
        pack0f[: arr.shape[0], c0 : c0 + ncol] = arr
    pack0f = np.ascontiguousarray(pack0f)
    parts2 = {
        "whht": whht,
        "mr": np.asarray(inputs["meas_r"], dtype=f32),
        "mi": np.asarray(inputs["meas_i"], dtype=f32),
    }
    pack2 = np.zeros((128, PACK2_COLS), f32)
    for name, (c0, ncol) in PACK2.items():
        arr = np.asarray(parts2[name], dtype=f32)
        pack2[: arr.shape[0], c0 : c0 + ncol] = arr
    pack2 = np.ascontiguousarray(pack2)

    shared = dict(
        lute=lute, lutp=lutp, pack0r=pack0r, pack0f=pack0f, pack1=pack1, pack2=pack2
    )
    in_maps = []
    for c in range(NCORES):
        bs = slice(BL * c, BL * (c + 1))
        # s-major position order: pos = s*BL + b
        wi_c = np.ascontiguousarray(wi_full[bs].T.reshape(NPOS, 1))
        au_c = np.ascontiguousarray(
            au_full[bs].transpose(1, 0, 2).reshape(NPOS, AUDIO)
        )
        m = dict(shared)
        m["wi"] = wi_c
        m["au"] = au_c
        in_maps.append(m)
    return in_maps


def kernel(**inputs):
    from concourse.bass_utils import run_bass_kernel_spmd

    nc = _get_nc()
    in_maps = make_in_maps(inputs)
    res = run_bass_kernel_spmd(nc, in_maps, core_ids=list(range(NCORES)))
    out = np.concatenate(
        [np.asarray(res.results[c]["y"]).reshape(BL, 1) for c in range(NCORES)], axis=0
    ).astype(np.float32)
    return out
